# revision 1
# baseline (speedup 1.0000x reference)
"""BlockDiagonalLowRankLinear Trainium2 kernel.

y = BlockDiag(blocks) @ x + U @ (V.T @ x), scaled by alpha, plus bias.

Shapes (full problem):
  x      [4, 2048, 4096] f32   -> flattened to [8192, 4096]
  blocks [16, 256, 256]  f32   (per-block [out, in])
  U      [4096, 64] f32, V [4096, 64] f32, bias [4096] f32, alpha [1] f32
  out    [4, 2048, 4096] f32

Sharding: data-parallel over tokens. Each of the 8 cores gets 1024 tokens
and the full (replicated) parameters; outputs are concatenated. No
collectives needed.

Per-core algorithm (T=1024 tokens, D=4096, R=64, NB=16, bi=bo=256):
  - Setup: stage params, PE-transpose blocks -> blocks^T and U -> U^T,
    scaling by alpha during the PSUM->SBUF rounding copies (f32r); round V
    to f32r; broadcast bias/alpha to all partitions with rank-1 matmuls.
  - Steady state, 4 slabs of 256 tokens, software-pipelined:
    Phase A(s): PE-transpose x to x^T (f32r) and accumulate t_lr = V^T x.
    Phase B(s): per (t-chunk, o-chunk) PSUM tile: U-term matmul (K=64,
      using alpha*U^T) + 4 block-diagonal matmuls (K=128); DVE adds bias,
      DMA writes out.  B(s) is interleaved with A(s+1) per o-chunk so the
      PE never drains.

All matmuls contract over the partition dim; x^T is produced with PE
transpose-mode (fp32, 2 cyc/row); compute matmuls are f32r (1 cyc/row at
free-dim >= 256). Bacc.compile() splits multi-waits into event semaphores.
"""

import numpy as np

import concourse.bacc as bacc
import concourse.bass as bass
import concourse.mybir as mybir
import concourse.tile as tile
from concourse.bass_utils import run_bass_kernel_spmd
from concourse.masks import make_identity

F32 = mybir.dt.float32
F32R = mybir.dt.float32r
BF16 = mybir.dt.bfloat16

N_CORES = 8
D = 4096          # in = out features
R = 64            # low rank
NB = 16           # diagonal blocks
BI = 256          # block in/out size
NK = D // 128     # 32 i-chunks
T_CORE = 1024     # tokens per core
T_SLAB = 256      # tokens per slab
OC = 512          # output column chunk


def build(t_core: int = T_CORE, repeats: int = 1):
    nc = bacc.Bacc("TRN2", target_bir_lowering=False, debug=False)
    x = nc.declare_dram_parameter("x", [t_core, D], F32R, isOutput=False)
    blocks = nc.declare_dram_parameter("blocks", [NB, BI, BI], F32R, isOutput=False)
    U = nc.declare_dram_parameter("U", [D, R], F32R, isOutput=False)
    V = nc.declare_dram_parameter("V", [D, R], F32, isOutput=False)
    bias = nc.declare_dram_parameter("bias", [D], F32, isOutput=False)
    alpha = nc.declare_dram_parameter("alpha", [1], F32, isOutput=False)
    out = nc.declare_dram_parameter("out", [t_core, D], F32, isOutput=True)

    n_slab = t_core // T_SLAB
    n_tc = T_SLAB // 128          # t-chunks per slab
    n_oc = D // OC                # 8 output chunks

    with tile.TileContext(nc) as tc:
        with (
            tc.tile_pool(name="const", bufs=1) as cpool,
            tc.tile_pool(name="psum", bufs=4, space="PSUM") as psum,
            tc.tile_pool(name="tpsum", bufs=3, space="PSUM") as tpsum,
            tc.tile_pool(name="lrpsum", bufs=1, space="PSUM") as lrpsum,
        ):
            xpool_cm = tc.tile_pool(name="xpool", bufs=2)
            xpool = xpool_cm.__enter__()
            xTpool_cm = tc.tile_pool(name="xT", bufs=2)
            xTpool = xTpool_cm.__enter__()
            opool_cm = tc.tile_pool(name="opool", bufs=3)
            opool = opool_cm.__enter__()
            spool_cm = tc.tile_pool(name="stage", bufs=1)
            spool = spool_cm.__enter__()

            ident_f32 = spool.tile([128, 128], F32)
            make_identity(nc, ident_f32[:])
            ident = cpool.tile([128, 128], F32R)
            nc.vector.tensor_copy(ident[:], ident_f32[:])

            # ---- DMAs first: x for slab 0, then params ----
            def load_xnat(s):
                t0 = s * T_SLAB
                tiles = []
                for tcI in range(n_tc):
                    xt = xpool.tile([128, D], F32R, tag="xnat")
                    for q in range(4):
                        nc.sync.dma_start(
                            xt[:, q * 1024:(q + 1) * 1024],
                            x[t0 + tcI * 128: t0 + (tcI + 1) * 128,
                              q * 1024:(q + 1) * 1024])
                    tiles.append(xt)
                return tiles

            ones_t = spool.tile([1, 128], F32)
            nc.vector.memset(ones_t[:], 1.0)
            # row selectors: sel[:, j, :] is [8, 128] with row j all-ones
            sel = spool.tile([8, 8, 128], F32)
            nc.gpsimd.memset(sel[:], 0.0)
            nc.gpsimd.affine_select(
                out=sel[:], in_=sel[:],
                compare_op=mybir.AluOpType.not_equal,
                fill=1.0, base=0, pattern=[[-1, 8], [0, 128]],
                channel_multiplier=1,
            )

            alpha_row = spool.tile([1, 1], F32)
            nc.sync.dma_start(alpha_row[:], alpha[None, :])
            bias_row = spool.tile([8, 512], F32)
            nc.sync.dma_start(bias_row[:], bias.rearrange("(r c) -> r c", r=8))

            xnat = load_xnat(0)

            v_stage = spool.tile([128, NK, R], F32, tag="uv")
            nc.sync.dma_start(v_stage[:], V.rearrange("(a p) r -> p a r", p=128))
            v_sb = cpool.tile([128, NK, R], BF16)
            nc.vector.tensor_copy(v_sb[:], v_stage[:])
            blk_view = blocks.rearrange("b (g p) i -> p (b g) i", p=128)

            # ---- steady-state phases (emitted interleaved below) ----
            xT_tiles = [None] * n_slab
            tlr_tiles = [None] * n_slab
            tlr_sb_tiles = [None] * n_slab

            def phaseA_transposes(s, oc, xnat_s):
                xT = xT_tiles[s]
                for pair in range(2):
                    ki0 = 4 * oc + 2 * pair
                    pt = tpsum.tile([128, 512], F32R, tag="tp")
                    for kk in range(2):
                        for tcI in range(n_tc):
                            nc.tensor.transpose(
                                pt[:, kk * 256 + tcI * 128: kk * 256 + (tcI + 1) * 128],
                                xnat_s[tcI][:, (ki0 + kk) * 128:(ki0 + kk + 1) * 128],
                                ident[:],
                            )
                    nc.scalar.copy(xT[:, ki0:ki0 + 2, :], pt[:])

            def phaseA_st1(s, oc):
                xT = xT_tiles[s]
                tlr = tlr_tiles[s]
                for kk in range(4):
                    ki = 4 * oc + kk
                    nc.tensor.matmul(
                        tlr[:], v_sb[:, ki, :], xT[:, ki, :],
                        start=(ki == 0), stop=(ki == NK - 1),
                        skip_group_check=True,
                    )

            def phaseA_group(s, oc, xnat_s):
                phaseA_transposes(s, oc, xnat_s)
                phaseA_st1(s, oc)

            def phaseA_start(s):
                xT_t = xTpool.tile([128, NK, T_SLAB], BF16, tag="xT")
                xT_tiles[s] = xT_t
                tlr_t = lrpsum.tile([R, T_SLAB], F32, tag="tlr")
                tlr_tiles[s] = tlr_t

            def phaseA_finish(s):
                tlr_sb = xTpool.tile([R, T_SLAB], BF16, tag="tlr_sb")
                nc.vector.tensor_copy(tlr_sb[:], tlr_tiles[s][:])
                tlr_sb_tiles[s] = tlr_sb

            def phaseB_bd(s, oc, blocksT):
                """block-diagonal matmuls for o-chunk oc of slab s"""
                xT = xT_tiles[s]
                accs = []
                for tcI in range(n_tc):
                    acc = psum.tile([128, OC], F32, tag="acc")
                    for kk in range(4):
                        ki = 4 * oc + kk
                        nc.tensor.matmul(
                            acc[:, (kk // 2) * 256:(kk // 2) * 256 + 256],
                            xT[:, ki, tcI * 128:(tcI + 1) * 128],
                            blocksT[:, ki, :],
                            start=(kk == 0), stop=False,
                            skip_group_check=True,
                        )
                    accs.append(acc)
                return accs

            def phaseB_fin(s, oc, accs, uT, bias_b):
                """U-term accumulate + bias + store for o-chunk oc of slab s"""
                t0 = (s % n_slab) * T_SLAB
                tlr_sb = tlr_sb_tiles[s]
                for tcI in range(n_tc):
                    acc = accs[tcI]
                    nc.tensor.matmul(
                        acc[:], tlr_sb[:, tcI * 128:(tcI + 1) * 128],
                        uT[:, 4 * oc:4 * oc + 4, :],
                        start=False, stop=True, skip_group_check=True,
                    )
                    o_sb = opool.tile([128, OC], F32, tag="osb")
                    nc.vector.tensor_tensor(
                        out=o_sb[:], in0=acc[:],
                        in1=bias_b[:, oc * OC:(oc + 1) * OC],
                        op=mybir.AluOpType.add,
                    )
                    nc.sync.dma_start(
                        out[t0 + tcI * 128: t0 + (tcI + 1) * 128,
                            oc * OC:(oc + 1) * OC],
                        o_sb[:],
                    )

            # ---- slab 0 Phase A (PE busy while params finish staging) ----
            phaseA_start(0)
            for oc in range(n_oc):
                phaseA_transposes(0, oc, xnat)
            for oc in range(n_oc):
                phaseA_st1(0, oc)
            phaseA_finish(0)

            # ---- param setup on PE (transpose-mode) + alpha-scaled copies ----
            # alpha broadcast to [128, 1]
            alpha_col = cpool.tile([128, 1], F32)
            a_ps = tpsum.tile([128, 512], F32, tag="tp")
            nc.tensor.matmul(a_ps[:, :1], ones_t[:], alpha_row[:],
                             start=True, stop=True)
            nc.vector.tensor_copy(alpha_col[:], a_ps[:, :1])

            blocksT = cpool.tile([128, NK, BI], BF16)
            u_stage = None
            uT = cpool.tile([R, NK, 128], BF16)
            bias_b = cpool.tile([128, D], F32)

            def setup_blocks_round(rnd):
                blk_stage = spool.tile([128, NB, BI], F32R, tag="blk")
                nc.sync.dma_start(blk_stage[:],
                                  blk_view[:, rnd * NB:(rnd + 1) * NB, :])
                for bb_ in range(NB // 2):
                    b = rnd * (NB // 2) + bb_
                    for ihalf in range(2):
                        ki = 2 * b + ihalf
                        pt = tpsum.tile([128, 512], F32R, tag="tp")
                        for g in range(2):
                            nc.tensor.transpose(
                                pt[:, g * 128:(g + 1) * 128],
                                blk_stage[:, 2 * bb_ + g, ihalf * 128:(ihalf + 1) * 128],
                                ident[:],
                            )
                        nc.vector.tensor_scalar_mul(blocksT[:, ki, :], pt[:, :256],
                                                    alpha_col[:, 0:1])

            setup_blocks_round(0)

            for j in range(n_oc):
                bp = tpsum.tile([128, 512], F32, tag="tp")
                nc.tensor.matmul(bp[:], sel[:, j, :], bias_row[:, :],
                                 start=True, stop=True)
                nc.vector.tensor_copy(bias_b[:, j * 512:(j + 1) * 512], bp[:])

            u_stage = spool.tile([128, NK, R], F32R, tag="uv")
            nc.sync.dma_start(u_stage[:], U.rearrange("(a p) r -> p a r", p=128))
            for j in range(NK // 4):
                up = tpsum.tile([128, 512], F32R, tag="tp")
                for q in range(4):
                    a = 4 * j + q
                    nc.tensor.transpose(
                        up[:R, q * 128:(q + 1) * 128], u_stage[:, a, :], ident[:])
                nc.vector.tensor_scalar_mul(uT[:, 4 * j:4 * j + 4, :], up[:R, :],
                                            alpha_col[:R, 0:1])

            setup_blocks_round(1)

            # ---- software-pipelined steady loop ----
            total = repeats * n_slab
            for it in range(total):
                s = it % n_slab
                nxt = it + 1
                if nxt < total:
                    xnat = load_xnat(nxt % n_slab)
                    phaseA_start(nxt % n_slab) if False else None
                # interleave B(s) with A(s+1)
                if nxt < total:
                    sn = nxt % n_slab
                    xT_t = xTpool.tile([128, NK, T_SLAB], BF16, tag="xT")
                    xT_tiles[sn] = xT_t
                    tlr_t = lrpsum.tile([R, T_SLAB], F32, tag="tlr")
                    tlr_tiles[sn] = tlr_t
                pending = None
                for oc in range(n_oc):
                    if nxt < total:
                        phaseA_group(nxt % n_slab, oc, xnat)
                    accs = phaseB_bd(s, oc, blocksT)
                    if pending is not None:
                        phaseB_fin(s, pending[0], pending[1], uT, bias_b)
                    pending = (oc, accs)
                phaseB_fin(s, pending[0], pending[1], uT, bias_b)
                if nxt < total:
                    phaseA_finish(nxt % n_slab)

            spool_cm.__exit__(None, None, None)
            opool_cm.__exit__(None, None, None)
            xTpool_cm.__exit__(None, None, None)
            xpool_cm.__exit__(None, None, None)
    nc.compile()
    return nc


def check_waits(nc, verbose=True):
    bad = 0
    for fn in nc.m.functions:
        for bb in fn.blocks:
            for ins in bb.instructions:
                tname = type(ins).__name__
                if tname == "InstDrain":
                    continue
                nw = len(ins.sync_info.on_wait) if ins.sync_info else 0
                if tname == "InstEventSemaphore" and nw <= 2:
                    continue
                if nw > 1:
                    bad += 1
                    if verbose:
                        print("MULTI-WAIT", tname, ins.name,
                              [(w.ant_name, w.wait_value) for w in ins.sync_info.on_wait])
    return bad


_NC_CACHE = {}


def _get_nc(t_core, repeats=1):
    key = (t_core, repeats)
    if key not in _NC_CACHE:
        _NC_CACHE[key] = build(t_core, repeats)
    return _NC_CACHE[key]


def kernel(x, blocks, U, V, bias, alpha):
    batch_dims = x.shape[:-1]
    x_flat = np.ascontiguousarray(x.reshape(-1, D).astype(np.float32))
    n_tok = x_flat.shape[0]
    t_core = n_tok // N_CORES
    nc = _get_nc(t_core)

    blocks = np.ascontiguousarray(blocks, dtype=np.float32)
    U = np.ascontiguousarray(U, dtype=np.float32)
    V = np.ascontiguousarray(V, dtype=np.float32)
    bias = np.ascontiguousarray(bias, dtype=np.float32)
    alpha = np.ascontiguousarray(alpha, dtype=np.float32)

    in_maps = [
        {
            "x": x_flat[c * t_core:(c + 1) * t_core],
            "blocks": blocks, "U": U, "V": V, "bias": bias, "alpha": alpha,
        }
        for c in range(N_CORES)
    ]
    res = run_bass_kernel_spmd(nc, in_maps, list(range(N_CORES)))
    out = np.concatenate([res.results[c]["out"] for c in range(N_CORES)], axis=0)
    return out.reshape(*batch_dims, D)



# revision 12
# speedup vs baseline: 12.5979x; 12.5979x over previous
"""BlockDiagonalLowRankLinear Trainium2 kernel.

y = BlockDiag(blocks) @ x + U @ (V.T @ x), scaled by alpha, plus bias.

Shapes (full problem):
  x      [4, 2048, 4096] f32   -> flattened to [8192, 4096]
  blocks [16, 256, 256]  f32   (per-block [out, in])
  U      [4096, 64] f32, V [4096, 64] f32, bias [4096] f32, alpha [1] f32
  out    [4, 2048, 4096] f32

Sharding: data-parallel over tokens. Each of the 8 cores gets 1024 tokens
and the full (replicated) parameters; outputs are concatenated. No
collectives needed.

Per-core algorithm (T=1024 tokens, D=4096, R=64, NB=16, bi=bo=256):
  Setup (once): stage params; PE-transpose blocks -> blocksT bf16 (alpha
  folded in) and U -> uTb bf16 [65, D] whose row 64 is the bias (alpha on
  rows 0..63 only); V -> v_sb bf16.
  Steady state, halves of H=512 tokens:
    - 8 input DMAs [128, 2048] (sync queue)
    - PE transpose x -> xT bf16 [128, NK, H]; PSUM->SBUF copies alternate
      DVE/ACT
    - V-term: 32 matmuls N=H into tlr [64, H]; copied to tsb [65, H] whose
      row 64 is constant 1.0 (so the U matmul adds bias via its 65th
      contraction row)
    - per token-chunk: 8 PSUM acc tiles [128, 512]: 4 block-diagonal
      matmuls (K=128 each) + one U matmul (K=65, moving uTb N=512,
      stop=True); acc copied to osb (DVE/ACT alternating); one 2MB output
      DMA per token-chunk on the scalar queue.
  The in-NEFF repeat loop is a hardware For_i loop (hint_engines=PE), so
  NEFF size is independent of the repeat count used for timing.
"""

import numpy as np

import concourse.bacc as bacc
import concourse.bass as bass
import concourse.mybir as mybir
import concourse.tile as tile
from concourse.bass_utils import run_bass_kernel_spmd
from concourse.masks import make_identity

F32 = mybir.dt.float32
F32R = mybir.dt.float32r
BF16 = mybir.dt.bfloat16

N_CORES = 8
D = 4096          # in = out features
R = 64            # low rank
NB = 16           # diagonal blocks
BI = 256          # block in/out size
NK = D // 128     # 32 i-chunks
T_CORE = 1024     # tokens per core
OC = 512          # output column chunk (one PSUM bank of f32)


def build(t_core: int = T_CORE, repeats: int = 1, io: str = "all"):
    do_in = io in ("all", "in")
    do_out = io in ("all", "out")
    nc = bacc.Bacc("TRN2", target_bir_lowering=False, debug=False)
    x = nc.declare_dram_parameter("x", [t_core, D], F32R, isOutput=False)
    blocks = nc.declare_dram_parameter("blocks", [NB, BI, BI], F32R, isOutput=False)
    U = nc.declare_dram_parameter("U", [D, R], F32R, isOutput=False)
    V = nc.declare_dram_parameter("V", [D, R], F32, isOutput=False)
    bias = nc.declare_dram_parameter("bias", [D], F32, isOutput=False)
    alpha = nc.declare_dram_parameter("alpha", [1], F32, isOutput=False)
    out = nc.declare_dram_parameter("out", [t_core, D], F32, isOutput=True)

    H = 512 if t_core % 512 == 0 else t_core   # tokens per half-pass
    n_h = t_core // H
    n_tc = H // 128               # 128-token chunks per half
    XC = 2048                     # input dma column chunk
    n_xc = D // XC

    with tile.TileContext(nc) as tc:
        with (
            tc.tile_pool(name="const", bufs=1) as cpool,
            tc.tile_pool(name="stage", bufs=1) as spool,
            tc.tile_pool(name="xnat", bufs=5) as xpool,
            tc.tile_pool(name="xT", bufs=2) as xTpool,
            tc.tile_pool(name="osb", bufs=2) as opool,
            tc.tile_pool(name="tsb", bufs=2) as tsbpool,
            tc.tile_pool(name="tp", bufs=3, space="PSUM") as tppool,
            tc.tile_pool(name="acc", bufs=3, space="PSUM") as accpool,
            tc.tile_pool(name="lr", bufs=2, space="PSUM") as lrpool,
        ):
            # ---------- constants ----------
            ident_f32 = spool.tile([128, 128], F32, tag="ident_f32")
            make_identity(nc, ident_f32[:])
            ident = cpool.tile([128, 128], F32R, tag="ident")
            nc.vector.tensor_copy(ident[:], ident_f32[:])

            ones_t = spool.tile([1, 128], F32, tag="ones")
            nc.vector.memset(ones_t[:], 1.0)
            alpha_row = spool.tile([1, 1], F32, tag="alpha_row")
            nc.sync.dma_start(alpha_row[:], alpha[None, :])
            # broadcast alpha to [128, 1] via rank-1 matmul
            alpha_col = cpool.tile([128, 1], F32, tag="alpha_col")
            a_ps = tppool.tile([128, 512], F32, tag="tp")
            nc.tensor.matmul(a_ps[:, :1], ones_t[:], alpha_row[:],
                             start=True, stop=True)
            nc.vector.tensor_copy(alpha_col[:], a_ps[:, :1])

            # ---------- params: blocksT, uTb(+bias), v_sb ----------
            blocksT = cpool.tile([128, NK, BI], BF16, tag="blocksT")
            uTb = cpool.tile([65, NK, 128], BF16, tag="uTb")
            v_sb = cpool.tile([128, NK, R], BF16, tag="v_sb")

            blk_view = blocks.rearrange("b (g p) i -> p (b g) i", p=128)

            def setup_blocks_round(rnd):
                blk_stage = spool.tile([128, NB, BI], F32R, tag="blk")
                nc.sync.dma_start(blk_stage[:],
                                  blk_view[:, rnd * NB:(rnd + 1) * NB, :])
                for bb_ in range(NB // 2):
                    b = rnd * (NB // 2) + bb_
                    for ihalf in range(2):
                        ki = 2 * b + ihalf
                        pt = tppool.tile([128, 512], F32R, tag="tp")
                        for g in range(2):
                            nc.tensor.transpose(
                                pt[:, g * 128:(g + 1) * 128],
                                blk_stage[:, 2 * bb_ + g,
                                          ihalf * 128:(ihalf + 1) * 128],
                                ident[:],
                            )
                        nc.vector.tensor_scalar_mul(
                            blocksT[:, ki, :], pt[:, :256], alpha_col[:, 0:1])

            setup_blocks_round(0)
            setup_blocks_round(1)

            v_stage = spool.tile([128, NK, R], F32, tag="uv")
            nc.sync.dma_start(v_stage[:], V.rearrange("(a p) r -> p a r", p=128))
            nc.vector.tensor_copy(v_sb[:], v_stage[:])

            u_stage = spool.tile([128, NK, R], F32R, tag="uv")
            nc.sync.dma_start(u_stage[:], U.rearrange("(a p) r -> p a r", p=128))
            for j in range(NK // 4):
                up = tppool.tile([128, 512], F32R, tag="tp")
                for q in range(4):
                    a = 4 * j + q
                    nc.tensor.transpose(
                        up[:R, q * 128:(q + 1) * 128], u_stage[:, a, :], ident[:])
                nc.vector.tensor_scalar_mul(
                    uTb[:R, 4 * j:4 * j + 4, :], up[:R, :], alpha_col[:R, 0:1])

            bias_row = spool.tile([1, NK, 128], F32, tag="blk")
            nc.sync.dma_start(bias_row[:], bias[None, :])
            nc.vector.tensor_copy(uTb[R:R + 1, :, :], bias_row[:])

            # ---------- steady state ----------
            rr = [0]

            def copy_rr(dst, src):
                if rr[0] % 2 == 0:
                    nc.vector.tensor_copy(dst, src)
                else:
                    nc.scalar.copy(dst, src)
                rr[0] += 1

            def one_pass():
                for h in range(n_h):
                    t0 = h * H
                    xts = []
                    for tcI in range(n_tc):
                        row = []
                        for q in range(n_xc):
                            xnat = xpool.tile([128, XC], F32R, tag="xnat")
                            if do_in or h == 0:
                                nc.sync.dma_start(
                                    xnat[:],
                                    x[t0 + tcI * 128: t0 + (tcI + 1) * 128,
                                      q * XC:(q + 1) * XC])
                            row.append(xnat)
                        xts.append(row)

                    xT = xTpool.tile([128, NK, H], BF16, tag="xT")
                    nkq = XC // 128           # ki chunks per xnat tile
                    for tcI in range(n_tc):
                        for g in range(NK // 4):
                            pt = tppool.tile([128, 512], F32R, tag="tp")
                            for q in range(4):
                                ki = 4 * g + q
                                src = xts[tcI][ki // nkq]
                                kk = ki % nkq
                                nc.tensor.transpose(
                                    pt[:, q * 128:(q + 1) * 128],
                                    src[:, kk * 128:(kk + 1) * 128],
                                    ident[:],
                                )
                            copy_rr(
                                xT[:, 4 * g:4 * g + 4,
                                   tcI * 128:(tcI + 1) * 128],
                                pt[:])

                    tlr = lrpool.tile([R, H], F32, tag="tlr")
                    for ki in range(NK):
                        nc.tensor.matmul(
                            tlr[:], v_sb[:, ki, :], xT[:, ki, :],
                            start=(ki == 0), stop=(ki == NK - 1),
                            skip_group_check=True,
                        )
                    tsb = tsbpool.tile([R + 1, H], BF16, tag="tsb")
                    nc.gpsimd.memset(tsb[R:R + 1, :], 1.0)
                    nc.vector.tensor_copy(tsb[:R, :], tlr[:])

                    for tcI in range(n_tc):
                        osb = opool.tile([128, D], F32, tag="osb")
                        for oc in range(D // OC):
                            acc = accpool.tile([128, OC], F32, tag="acc")
                            for b2 in range(2):
                                b = 2 * oc + b2
                                for ih in range(2):
                                    ki = 2 * b + ih
                                    nc.tensor.matmul(
                                        acc[:, b2 * 256:(b2 + 1) * 256],
                                        xT[:, ki, tcI * 128:(tcI + 1) * 128],
                                        blocksT[:, ki, :],
                                        start=(b2 == 0 and ih == 0), stop=False,
                                        skip_group_check=True,
                                    )
                            nc.tensor.matmul(
                                acc[:], tsb[:, tcI * 128:(tcI + 1) * 128],
                                uTb[:, 4 * oc:4 * oc + 4, :],
                                start=False, stop=True, skip_group_check=True,
                            )
                            copy_rr(osb[:, oc * OC:(oc + 1) * OC], acc[:])
                        if do_out:
                            nc.scalar.dma_start(
                                out[t0 + tcI * 128: t0 + (tcI + 1) * 128, :],
                                osb[:])

            if repeats == 1:
                one_pass()
            else:
                with tc.For_i(0, repeats, 1,
                              hint_engines=(mybir.EngineType.PE,)):
                    one_pass()
    nc.compile()
    return nc


def check_waits(nc, verbose=True):
    bad = 0
    for fn in nc.m.functions:
        for bb in fn.blocks:
            for ins in bb.instructions:
                tname = type(ins).__name__
                if tname == "InstDrain":
                    continue
                nw = len(ins.sync_info.on_wait) if ins.sync_info else 0
                if tname == "InstEventSemaphore" and nw <= 2:
                    continue
                if nw > 1:
                    bad += 1
                    if verbose:
                        print("MULTI-WAIT", tname, ins.name,
                              [(w.ant_name, w.wait_value) for w in ins.sync_info.on_wait])
    return bad


_NC_CACHE = {}


def _get_nc(t_core, repeats=1):
    key = (t_core, repeats)
    if key not in _NC_CACHE:
        _NC_CACHE[key] = build(t_core, repeats)
    return _NC_CACHE[key]


def kernel(x, blocks, U, V, bias, alpha):
    batch_dims = x.shape[:-1]
    x_flat = np.ascontiguousarray(x.reshape(-1, D).astype(np.float32))
    n_tok = x_flat.shape[0]
    t_core = n_tok // N_CORES
    nc = _get_nc(t_core)

    blocks = np.ascontiguousarray(blocks, dtype=np.float32)
    U = np.ascontiguousarray(U, dtype=np.float32)
    V = np.ascontiguousarray(V, dtype=np.float32)
    bias = np.ascontiguousarray(bias, dtype=np.float32)
    alpha = np.ascontiguousarray(alpha, dtype=np.float32)

    in_maps = [
        {
            "x": x_flat[c * t_core:(c + 1) * t_core],
            "blocks": blocks, "U": U, "V": V, "bias": bias, "alpha": alpha,
        }
        for c in range(N_CORES)
    ]
    res = run_bass_kernel_spmd(nc, in_maps, list(range(N_CORES)))
    out = np.concatenate([res.results[c]["out"] for c in range(N_CORES)], axis=0)
    return out.reshape(*batch_dims, D)


# revision 17
# speedup vs baseline: 13.7267x; 1.0896x over previous
"""BlockDiagonalLowRankLinear Trainium2 kernel.

y = BlockDiag(blocks) @ x + U @ (V.T @ x), scaled by alpha, plus bias.

Shapes (full problem):
  x      [4, 2048, 4096] f32   -> flattened to [8192, 4096]
  blocks [16, 256, 256]  f32   (per-block [out, in])
  U      [4096, 64] f32, V [4096, 64] f32, bias [4096] f32, alpha [1] f32
  out    [4, 2048, 4096] f32

Sharding: data-parallel over tokens. Each of the 8 cores gets 1024 tokens
and the full (replicated) parameters; outputs are concatenated. No
collectives needed.

Per-core algorithm (T=1024 tokens, D=4096, R=64, NB=16, bi=bo=256):
  Setup (once): stage params; PE-transpose blocks -> blocksT bf16 (alpha
  folded in) and U -> uTb bf16 [65, D] whose row 64 is the bias (alpha on
  rows 0..63 only); V -> v_sb bf16.
  Steady state, halves of H=512 tokens:
    - 8 input DMAs [128, 2048] (sync queue)
    - PE transpose x -> xT bf16 [128, NK, H]; PSUM->SBUF copies alternate
      DVE/ACT
    - V-term: 32 matmuls N=H into tlr [64, H]; copied to tsb [65, H] whose
      row 64 is constant 1.0 (so the U matmul adds bias via its 65th
      contraction row)
    - per token-chunk: 8 PSUM acc tiles [128, 512]: 4 block-diagonal
      matmuls (K=128 each) + one U matmul (K=65, moving uTb N=512,
      stop=True); acc copied to osb (DVE/ACT alternating); one 2MB output
      DMA per token-chunk on the scalar queue.
  The in-NEFF repeat loop is a hardware For_i loop (hint_engines=PE), so
  NEFF size is independent of the repeat count used for timing.
"""

import numpy as np

import concourse.bacc as bacc
import concourse.bass as bass
import concourse.mybir as mybir
import concourse.tile as tile
from concourse.bass_utils import run_bass_kernel_spmd
from concourse.masks import make_identity

F32 = mybir.dt.float32
F32R = mybir.dt.float32r
BF16 = mybir.dt.bfloat16

N_CORES = 8
D = 4096          # in = out features
R = 64            # low rank
NB = 16           # diagonal blocks
BI = 256          # block in/out size
NK = D // 128     # 32 i-chunks
T_CORE = 1024     # tokens per core
OC = 512          # output column chunk (one PSUM bank of f32)


def build(t_core: int = T_CORE, repeats: int = 1, io: str = "all",
          copy_mod: int = 2, copy_thresh: int = 1, out_q: str = "scalar",
          acc_bufs: int = 3, lr_bufs: int = 2, staggered: bool = False):
    do_in = io in ("all", "in")
    do_out = io in ("all", "out")
    nc = bacc.Bacc("TRN2", target_bir_lowering=False, debug=False)
    x = nc.declare_dram_parameter("x", [t_core, D], F32R, isOutput=False)
    blocks = nc.declare_dram_parameter("blocks", [NB, BI, BI], F32R, isOutput=False)
    U = nc.declare_dram_parameter("U", [D, R], F32R, isOutput=False)
    V = nc.declare_dram_parameter("V", [D, R], F32, isOutput=False)
    bias = nc.declare_dram_parameter("bias", [D], F32, isOutput=False)
    alpha = nc.declare_dram_parameter("alpha", [1], F32, isOutput=False)
    out = nc.declare_dram_parameter("out", [t_core, D], F32, isOutput=True)

    H = 512 if t_core % 512 == 0 else t_core   # tokens per half-pass
    n_h = t_core // H
    n_tc = H // 128               # 128-token chunks per half
    XC = 2048                     # input dma column chunk
    n_xc = D // XC

    with tile.TileContext(nc) as tc:
        with (
            tc.tile_pool(name="const", bufs=1) as cpool,
            tc.tile_pool(name="stage", bufs=1) as spool,
            tc.tile_pool(name="xnat", bufs=5) as xpool,
            tc.tile_pool(name="xT", bufs=2) as xTpool,
            tc.tile_pool(name="osb", bufs=2) as opool,
            tc.tile_pool(name="tsb", bufs=2) as tsbpool,
            tc.tile_pool(name="tp", bufs=3, space="PSUM") as tppool,
            tc.tile_pool(name="acc", bufs=acc_bufs, space="PSUM") as accpool,
            tc.tile_pool(name="lr", bufs=lr_bufs, space="PSUM") as lrpool,
        ):
            # ---------- constants ----------
            ident_f32 = spool.tile([128, 128], F32, tag="ident_f32")
            make_identity(nc, ident_f32[:])
            ident = cpool.tile([128, 128], F32R, tag="ident")
            nc.vector.tensor_copy(ident[:], ident_f32[:])

            ones_t = spool.tile([1, 128], F32, tag="ones")
            nc.vector.memset(ones_t[:], 1.0)
            alpha_row = spool.tile([1, 1], F32, tag="alpha_row")
            nc.sync.dma_start(alpha_row[:], alpha[None, :])
            # broadcast alpha to [128, 1] via rank-1 matmul
            alpha_col = cpool.tile([128, 1], F32, tag="alpha_col")
            a_ps = tppool.tile([128, 512], F32, tag="tp")
            nc.tensor.matmul(a_ps[:, :1], ones_t[:], alpha_row[:],
                             start=True, stop=True)
            nc.vector.tensor_copy(alpha_col[:], a_ps[:, :1])

            # ---------- params: blocksT, uTb(+bias), v_sb ----------
            blocksT = cpool.tile([128, NK, BI], BF16, tag="blocksT")
            uTb = cpool.tile([65, NK, 128], BF16, tag="uTb")
            v_sb = cpool.tile([128, NK, R], BF16, tag="v_sb")

            blk_view = blocks.rearrange("b (g p) i -> p (b g) i", p=128)

            def setup_blocks_round(rnd):
                blk_stage = spool.tile([128, NB, BI], F32R, tag="blk")
                nc.sync.dma_start(blk_stage[:],
                                  blk_view[:, rnd * NB:(rnd + 1) * NB, :])
                for bb_ in range(NB // 2):
                    b = rnd * (NB // 2) + bb_
                    for ihalf in range(2):
                        ki = 2 * b + ihalf
                        pt = tppool.tile([128, 512], F32R, tag="tp")
                        for g in range(2):
                            nc.tensor.transpose(
                                pt[:, g * 128:(g + 1) * 128],
                                blk_stage[:, 2 * bb_ + g,
                                          ihalf * 128:(ihalf + 1) * 128],
                                ident[:],
                            )
                        nc.vector.tensor_scalar_mul(
                            blocksT[:, ki, :], pt[:, :256], alpha_col[:, 0:1])

            setup_blocks_round(0)
            setup_blocks_round(1)

            v_stage = spool.tile([128, NK, R], F32, tag="uv")
            nc.sync.dma_start(v_stage[:], V.rearrange("(a p) r -> p a r", p=128))
            nc.vector.tensor_copy(v_sb[:], v_stage[:])

            u_stage = spool.tile([128, NK, R], F32R, tag="uv")
            nc.sync.dma_start(u_stage[:], U.rearrange("(a p) r -> p a r", p=128))
            for j in range(NK // 4):
                up = tppool.tile([128, 512], F32R, tag="tp")
                for q in range(4):
                    a = 4 * j + q
                    nc.tensor.transpose(
                        up[:R, q * 128:(q + 1) * 128], u_stage[:, a, :], ident[:])
                nc.vector.tensor_scalar_mul(
                    uTb[:R, 4 * j:4 * j + 4, :], up[:R, :], alpha_col[:R, 0:1])

            bias_row = spool.tile([1, NK, 128], F32, tag="blk")
            nc.sync.dma_start(bias_row[:], bias[None, :])
            nc.vector.tensor_copy(uTb[R:R + 1, :, :], bias_row[:])

            # ---------- steady state ----------
            rr = [0]

            def copy_rr(dst, src):
                if rr[0] % copy_mod < copy_thresh:
                    nc.vector.tensor_copy(dst, src)
                else:
                    nc.scalar.copy(dst, src)
                rr[0] += 1

            def one_pass():
                for h in range(n_h):
                    t0 = h * H
                    xts = []
                    for tcI in range(n_tc):
                        row = []
                        for q in range(n_xc):
                            xnat = xpool.tile([128, XC], F32R, tag="xnat")
                            if do_in or h == 0:
                                nc.sync.dma_start(
                                    xnat[:],
                                    x[t0 + tcI * 128: t0 + (tcI + 1) * 128,
                                      q * XC:(q + 1) * XC])
                            row.append(xnat)
                        xts.append(row)

                    xT = xTpool.tile([128, NK, H], BF16, tag="xT")
                    nkq = XC // 128           # ki chunks per xnat tile
                    for tcI in range(n_tc):
                        for g in range(NK // 4):
                            pt = tppool.tile([128, 512], F32R, tag="tp")
                            for q in range(4):
                                ki = 4 * g + q
                                src = xts[tcI][ki // nkq]
                                kk = ki % nkq
                                nc.tensor.transpose(
                                    pt[:, q * 128:(q + 1) * 128],
                                    src[:, kk * 128:(kk + 1) * 128],
                                    ident[:],
                                )
                            copy_rr(
                                xT[:, 4 * g:4 * g + 4,
                                   tcI * 128:(tcI + 1) * 128],
                                pt[:])

                    tlr = lrpool.tile([R, H], F32, tag="tlr")
                    for ki in range(NK):
                        nc.tensor.matmul(
                            tlr[:], v_sb[:, ki, :], xT[:, ki, :],
                            start=(ki == 0), stop=(ki == NK - 1),
                            skip_group_check=True,
                        )
                    tsb = tsbpool.tile([R + 1, H], BF16, tag="tsb")
                    nc.gpsimd.memset(tsb[R:R + 1, :], 1.0)
                    nc.vector.tensor_copy(tsb[:R, :], tlr[:])

                    for tcI in range(n_tc):
                        osb = opool.tile([128, D], F32, tag="osb")
                        for oc in range(D // OC):
                            acc = accpool.tile([128, OC], F32, tag="acc")
                            for b2 in range(2):
                                b = 2 * oc + b2
                                for ih in range(2):
                                    ki = 2 * b + ih
                                    nc.tensor.matmul(
                                        acc[:, b2 * 256:(b2 + 1) * 256],
                                        xT[:, ki, tcI * 128:(tcI + 1) * 128],
                                        blocksT[:, ki, :],
                                        start=(b2 == 0 and ih == 0), stop=False,
                                        skip_group_check=True,
                                    )
                            nc.tensor.matmul(
                                acc[:], tsb[:, tcI * 128:(tcI + 1) * 128],
                                uTb[:, 4 * oc:4 * oc + 4, :],
                                start=False, stop=True, skip_group_check=True,
                            )
                            copy_rr(osb[:, oc * OC:(oc + 1) * OC], acc[:])
                        if do_out:
                            oeng = nc.scalar if out_q == "scalar" else nc.sync
                            oeng.dma_start(
                                out[t0 + tcI * 128: t0 + (tcI + 1) * 128, :],
                                osb[:])

            if repeats == 1:
                one_pass()
            else:
                with tc.For_i(0, repeats, 1,
                              hint_engines=(mybir.EngineType.PE,),
                              staggered_reset=staggered):
                    one_pass()
    nc.compile()
    return nc


def check_waits(nc, verbose=True):
    bad = 0
    for fn in nc.m.functions:
        for bb in fn.blocks:
            for ins in bb.instructions:
                tname = type(ins).__name__
                if tname == "InstDrain":
                    continue
                nw = len(ins.sync_info.on_wait) if ins.sync_info else 0
                if tname == "InstEventSemaphore" and nw <= 2:
                    continue
                if nw > 1:
                    bad += 1
                    if verbose:
                        print("MULTI-WAIT", tname, ins.name,
                              [(w.ant_name, w.wait_value) for w in ins.sync_info.on_wait])
    return bad


_NC_CACHE = {}


def _get_nc(t_core, repeats=1):
    key = (t_core, repeats)
    if key not in _NC_CACHE:
        _NC_CACHE[key] = build(t_core, repeats)
    return _NC_CACHE[key]


def kernel(x, blocks, U, V, bias, alpha):
    batch_dims = x.shape[:-1]
    x_flat = np.ascontiguousarray(x.reshape(-1, D).astype(np.float32))
    n_tok = x_flat.shape[0]
    t_core = n_tok // N_CORES
    nc = _get_nc(t_core)

    blocks = np.ascontiguousarray(blocks, dtype=np.float32)
    U = np.ascontiguousarray(U, dtype=np.float32)
    V = np.ascontiguousarray(V, dtype=np.float32)
    bias = np.ascontiguousarray(bias, dtype=np.float32)
    alpha = np.ascontiguousarray(alpha, dtype=np.float32)

    in_maps = [
        {
            "x": x_flat[c * t_core:(c + 1) * t_core],
            "blocks": blocks, "U": U, "V": V, "bias": bias, "alpha": alpha,
        }
        for c in range(N_CORES)
    ]
    res = run_bass_kernel_spmd(nc, in_maps, list(range(N_CORES)))
    out = np.concatenate([res.results[c]["out"] for c in range(N_CORES)], axis=0)
    return out.reshape(*batch_dims, D)
